# revision 28
# baseline (speedup 1.0000x reference)
"""Trainium2 Bass kernel for MimickedSelfContactLoss (retrieval_knn).

Math reduction: the reference builds the full N x N vertex distance matrix but
only ever reads it at (contact, contact) index pairs, and the argmin feeds a
gather of the *same* distance matrix, so

    loss = mean_i tanh( min_{j : geomask[pc_i, pc_j]} ||v[pc_i] - v[pc_j]|| )

i.e. a C x C (1024 x 1024) masked-min pairwise-distance problem over the
contact subset.  (If a row has no allowed neighbor the reference would pick
column 0; with a ~0.5-dense random mask over 1024 columns that case has
probability ~2^-1024 and is ignored.)

Distribution: row-shard the C x C computation across 8 NeuronCores -- each
core owns 128 query contacts vs all 1024 contacts (the sharding_hint's
row-wise split applied to the contact subset, with its geomask rows sharded
alongside).  Per core:

  PE   : squared distances via K=5 fp32 matmuls using the
         |q|^2 + |k|^2 - 2 q.k expansion ([-2q; q^2; 1]^T [k; 1; k^2])
  DVE  : score = dist2 + penalty  (penalty = BIG*(1-mask), exact 0/BIG in
         bf16), then min-reduce over the 1024 columns -> [128,1];
         threshold at TAU restores the exact zeros that fp32 cancellation in
         the matmul expansion loses (residual < 1e-5 << TAU << 2.4e-4 = the
         smallest genuine nonzero contact dist^2)
  ACT  : sqrt, tanh -> [128,1]
  PE   : dot with ones -> per-core sum of tanh as a single [1,1] value (a
         128-partition output DMA costs ~4us in per-descriptor/semaphore
         overhead; a single-packet scalar is ~free)

Hand-scheduled raw bacc (no TileContext; Tile's barrier machinery costs more
than the compute).  Scheduling notes baked in below:
  - input DMAs are split across all three DMA-capable queues (sync HWDGE,
    scalar HWDGE, gpsimd SWDGE) -- a single dynamic queue moves only
    ~25 GB/s with 2KB row packets
  - a dummy sqrt at t=0 preloads the sqrt LUT under the DMAs; a dummy tanh
    right after the real sqrt starts the exp-table load while DVE finishes,
    so the swap is (mostly) off the critical path
  - engines are deeply pipelined, so every same-engine RAW hazard carries an
    explicit semaphore wait

The 8 cores return their tanh-sum; the host adds them up (the "all-gather")
and divides by C.
"""

from contextlib import ExitStack

import numpy as np
import ml_dtypes

import concourse.bass as bass
import concourse.mybir as mybir
from concourse import bacc
from concourse.bass_utils import run_bass_kernel_spmd

N = 6890
C = 1024
NCORES = 8
P = C // NCORES          # 128 query rows per core
NCH = 2                  # free-dim chunks (fp32 matmul moving operand <= 512)
CH = C // NCH
BIG = float(2 ** 20)     # exact in bf16/f32; >> max contact dist^2 (~70)
TAU = 3e-5               # separates fp32 cancellation residue from real dist^2

# pen row ranges per DMA queue: [sync, scalar, gpsimd]
ROW_SPLIT = (28, 80)


def build_nc() -> bass.Bass:
    nc = bacc.Bacc("TRN2", target_bir_lowering=False, debug=False,
                   dynamic_dma_scratch_size=2048)
    dt = mybir.dt
    AX = mybir.AxisListType
    OP = mybir.AluOpType
    AF = mybir.ActivationFunctionType

    # aug packs [aq | ak]: cols 0:P the query block (lhsT), cols P:P+C the keys
    aug = nc.dram_tensor("aug", [5, P + C], dt.float32, kind="ExternalInput").ap()
    pen = nc.dram_tensor("pen", [P, C], dt.uint8, kind="ExternalInput").ap()
    out = nc.dram_tensor("out", [1, 1], dt.float32, kind="ExternalOutput").ap()

    with ExitStack() as ctx:
        en = ctx.enter_context
        aug_s = en(nc.sbuf_tensor("aug_s", [5, P + C], dt.float32))
        pen_s = en(nc.sbuf_tensor("pen_s", [P, C], dt.uint8))
        scr = en(nc.sbuf_tensor("scr", [P, C], dt.float32))
        # stat columns: 0..1 chunk mins | 2 min | 3 thr | 4 m2 | 5 v | 6 scratch | 7 tanh
        stat = en(nc.sbuf_tensor("stat", [P, 10], dt.float32))
        ones = en(nc.sbuf_tensor("ones", [P, 1], dt.float32))
        warm = en(nc.sbuf_tensor("warm", [1, 640], dt.bfloat16))
        res = en(nc.sbuf_tensor("res", [1, 1], dt.float32))
        psum = [
            en(nc.psum_tensor(f"ps{i}", [P, CH], dt.float32)) for i in range(NCH)
        ]
        sum_ps = en(nc.psum_tensor("sum_ps", [1, 1], dt.float32))
        warm_ps = en(nc.psum_tensor("warm_ps", [P, CH], dt.float32))

        sem_ones = en(nc.semaphore("sem_ones"))
        sem_warm = en(nc.semaphore("sem_warm"))
        sem_w = en(nc.semaphore("sem_w"))      # PE warm-loop self ordering
        sem_aug = en(nc.semaphore("sem_aug"))
        sem_pen = en(nc.semaphore("sem_pen"))
        sem_pen2 = en(nc.semaphore("sem_pen2"))   # SWDGE sems must be exclusive
        sem_pe = en(nc.semaphore("sem_pe"))
        sem_v = en(nc.semaphore("sem_v"))      # DVE same-engine RAW ordering
        sem_a = en(nc.semaphore("sem_a"))      # ACT same-engine RAW ordering
        sem_dve = en(nc.semaphore("sem_dve"))
        sem_act = en(nc.semaphore("sem_act"))
        sem_sum = en(nc.semaphore("sem_sum"))
        sem_res = en(nc.semaphore("sem_res"))
        sem_out = en(nc.semaphore("sem_out"))
        block = en(nc.Block())

        @block.sync
        def _(s):
            s.dma_start(aug_s[0:3, :], aug[0:3, :]).then_inc(sem_aug, 16)
            s.dma_start(pen_s[0 : ROW_SPLIT[0], :], pen[0 : ROW_SPLIT[0], :]).then_inc(
                sem_pen, 16
            )
            s.wait_ge(sem_res, 1)
            s.dma_start(out[:], res[:]).then_inc(sem_out, 16)
            s.wait_ge(sem_out, 16)

        @block.scalar
        def _(a):
            a.dma_start(aug_s[3:5, :], aug[3:5, :]).then_inc(sem_aug, 16)
            a.dma_start(
                pen_s[ROW_SPLIT[0] : ROW_SPLIT[1], :],
                pen[ROW_SPLIT[0] : ROW_SPLIT[1], :],
            ).then_inc(sem_pen, 16)
            # dummy sqrt: pulls the sqrt LUT load to t~0, hidden under the DMAs
            a.wait_ge(sem_ones, 1)
            a.sqrt(stat[0:1, 8:9], ones[0:1, :])
            a.wait_ge(sem_dve, 1)
            a.sqrt(stat[:, 5:6], stat[:, 4:5]).then_inc(sem_a, 1)
            a.wait_ge(sem_a, 1)
            a.activation(stat[:, 7:8], stat[:, 5:6], AF.Tanh).then_inc(sem_act, 1)

        @block.gpsimd
        def _(g):
            g.memset(warm[:], 0.0).then_inc(sem_warm, 1)
            g.dma_start(
                pen_s[ROW_SPLIT[1] : P, :], pen[ROW_SPLIT[1] : P, :]
            ).then_inc(sem_pen2, 16)

        @block.tensor
        def _(t):
            # warm-up: ~2.8us of dummy bf16 matmuls flips the HAM clock gate
            # (1.2 -> 2.4 GHz) before the fp32 passes; PE is otherwise idle
            # while the aug DMA lands
            t.wait_ge(sem_warm, 1)
            for w in range(6):
                t.matmul(
                    warm_ps[:], warm[0:1, 0:P], warm[0:1, P : P + CH],
                    start=True, stop=True,
                ).then_inc(sem_w, 1)
                t.wait_ge(sem_w, w + 1)
            t.wait_ge(sem_aug, 32)
            for ch in range(NCH):
                t.matmul(
                    psum[ch][:], aug_s[:, 0:P],
                    aug_s[:, P + ch * CH : P + (ch + 1) * CH],
                    start=True, stop=True,
                ).then_inc(sem_pe, 1)
            t.wait_ge(sem_ones, 1)
            t.wait_ge(sem_act, 1)
            t.matmul(
                sum_ps[:], stat[:, 7:8], ones[:], start=True, stop=True
            ).then_inc(sem_sum, 1)

        @block.vector
        def _(v):
            c = 0
            v.memset(ones[:], 1.0).then_inc(sem_ones, 1)
            v.wait_ge(sem_pen, 32)
            v.wait_ge(sem_pen2, 16)
            for ch in range(NCH):
                sl = bass.ts(ch, CH)
                v.wait_ge(sem_pe, ch + 1)
                v.tensor_tensor(
                    out=scr[:, sl], in0=psum[ch][:], in1=pen_s[:, sl], op=OP.add
                ).then_inc(sem_v, 1)
                c += 1
                v.wait_ge(sem_v, c)
                v.tensor_reduce(
                    stat[:, ch : ch + 1], scr[:, sl], axis=AX.X, op=OP.min
                ).then_inc(sem_v, 1)
                c += 1
            v.wait_ge(sem_v, c)
            v.tensor_reduce(
                stat[:, 2:3], stat[:, 0:NCH], axis=AX.X, op=OP.min
            ).then_inc(sem_v, 1)
            c += 1
            v.wait_ge(sem_v, c)
            v.tensor_scalar(
                out=stat[:, 3:4], in0=stat[:, 2:3], scalar1=TAU, scalar2=None,
                op0=OP.is_ge,
            ).then_inc(sem_v, 1)
            c += 1
            v.wait_ge(sem_v, c)
            v.tensor_tensor(
                out=stat[:, 4:5], in0=stat[:, 2:3], in1=stat[:, 3:4], op=OP.mult
            ).then_inc(sem_dve, 1)
            v.wait_ge(sem_sum, 1)
            v.tensor_copy(res[:], sum_ps[:]).then_inc(sem_res, 1)

    nc.compile()
    return nc


def prepare_in_maps(presented_contact, vertices, geomask):
    pc = np.asarray(presented_contact).astype(np.int64)
    verts = np.asarray(vertices, dtype=np.float32).reshape(N, 3)
    gm = np.asarray(geomask)

    vc = verts[pc]                                    # [C, 3]
    q2 = (vc * vc).sum(axis=1, dtype=np.float32)      # [C]
    ones = np.ones((1, C), np.float32)
    ak = np.concatenate([vc.T, ones, q2[None, :]], axis=0).astype(np.float32)
    mg = gm[pc][:, pc]                                # [C, C] bool
    pen = np.where(mg, 0, 255).astype(np.uint8)   # 255 > max contact dist^2

    in_maps = []
    for g in range(NCORES):
        sl = slice(g * P, (g + 1) * P)
        aq = np.concatenate(
            [(-2.0 * vc[sl].T), q2[None, sl], np.ones((1, P), np.float32)], axis=0
        ).astype(np.float32)
        aug = np.concatenate([aq, ak], axis=1).astype(np.float32)   # [5, P+C]
        in_maps.append({
            "aug": np.ascontiguousarray(aug),
            "pen": np.ascontiguousarray(pen[sl]),
        })
    return in_maps


def finish(results) -> np.ndarray:
    sums = np.array([results[g]["out"][0, 0] for g in range(NCORES)], np.float64)
    return np.asarray(sums.sum() / C, dtype=np.float32)


def kernel(presented_contact, vertices, geomask) -> np.ndarray:
    in_maps = prepare_in_maps(presented_contact, vertices, geomask)
    nc = build_nc()
    res = run_bass_kernel_spmd(nc, in_maps, list(range(NCORES)))
    return finish(res.results)


# revision 29
# speedup vs baseline: 1.0531x; 1.0531x over previous
"""Trainium2 Bass kernel for MimickedSelfContactLoss (retrieval_knn).

Math reduction: the reference builds the full N x N vertex distance matrix but
only ever reads it at (contact, contact) index pairs, and the argmin feeds a
gather of the *same* distance matrix, so

    loss = mean_i tanh( min_{j : geomask[pc_i, pc_j]} ||v[pc_i] - v[pc_j]|| )

i.e. a C x C (1024 x 1024) masked-min pairwise-distance problem over the
contact subset.  (If a row has no allowed neighbor the reference would pick
column 0; with a ~0.5-dense random mask over 1024 columns that case has
probability ~2^-1024 and is ignored.)

Distribution: row-shard the C x C computation across 8 NeuronCores -- each
core owns 128 query contacts vs all 1024 contacts (the sharding_hint's
row-wise split applied to the contact subset, with its geomask rows sharded
alongside).  Per core:

  PE   : squared distances via K=5 fp32 matmuls using the
         |q|^2 + |k|^2 - 2 q.k expansion ([-2q; q^2; 1]^T [k; 1; k^2])
  DVE  : score = dist2 + penalty  (penalty = BIG*(1-mask), exact 0/BIG in
         bf16), then min-reduce over the 1024 columns -> [128,1];
         threshold at TAU restores the exact zeros that fp32 cancellation in
         the matmul expansion loses (residual < 1e-5 << TAU << 2.4e-4 = the
         smallest genuine nonzero contact dist^2)
  ACT  : sqrt, tanh -> [128,1]
  PE   : dot with ones -> per-core sum of tanh as a single [1,1] value (a
         128-partition output DMA costs ~4us in per-descriptor/semaphore
         overhead; a single-packet scalar is ~free)

Hand-scheduled raw bacc (no TileContext; Tile's barrier machinery costs more
than the compute).  Scheduling notes baked in below:
  - input DMAs are split across all three DMA-capable queues (sync HWDGE,
    scalar HWDGE, gpsimd SWDGE) -- a single dynamic queue moves only
    ~25 GB/s with 2KB row packets
  - a dummy sqrt at t=0 preloads the sqrt LUT under the DMAs; a dummy tanh
    right after the real sqrt starts the exp-table load while DVE finishes,
    so the swap is (mostly) off the critical path
  - engines are deeply pipelined, so every same-engine RAW hazard carries an
    explicit semaphore wait

The 8 cores return their tanh-sum; the host adds them up (the "all-gather")
and divides by C.
"""

from contextlib import ExitStack

import numpy as np
import ml_dtypes

import concourse.bass as bass
import concourse.mybir as mybir
from concourse import bacc
from concourse.bass_utils import run_bass_kernel_spmd

N = 6890
C = 1024
NCORES = 8
P = C // NCORES          # 128 query rows per core
NCH = 2                  # free-dim chunks (fp32 matmul moving operand <= 512)
CH = C // NCH
BIG = float(2 ** 20)     # exact in bf16/f32; >> max contact dist^2 (~70)
TAU = 3e-5               # separates fp32 cancellation residue from real dist^2

# pen row ranges per DMA queue: [sync, scalar, gpsimd]
ROW_SPLIT = (28, 80)


def build_nc() -> bass.Bass:
    nc = bacc.Bacc("TRN2", target_bir_lowering=False, debug=False,
                   dynamic_dma_scratch_size=2048)
    dt = mybir.dt
    AX = mybir.AxisListType
    OP = mybir.AluOpType
    AF = mybir.ActivationFunctionType

    # aug packs [aq | ak]: cols 0:P the query block (lhsT), cols P:P+C the keys
    aug = nc.dram_tensor("aug", [5, P + C], dt.float32, kind="ExternalInput").ap()
    pen = nc.dram_tensor("pen", [P, C], dt.uint8, kind="ExternalInput").ap()
    out = nc.dram_tensor("out", [1, 1], dt.float32, kind="ExternalOutput").ap()

    with ExitStack() as ctx:
        en = ctx.enter_context
        aug_s = en(nc.sbuf_tensor("aug_s", [5, P + C], dt.float32))
        pen_s = en(nc.sbuf_tensor("pen_s", [P, C], dt.uint8))
        scr = en(nc.sbuf_tensor("scr", [P, C], dt.float32))
        # stat columns: 0..1 chunk mins | 2 min | 3 thr | 4 m2 | 5 v | 6 scratch | 7 tanh
        stat = en(nc.sbuf_tensor("stat", [P, 10], dt.float32))
        ones = en(nc.sbuf_tensor("ones", [P, 1], dt.float32))
        warm = en(nc.sbuf_tensor("warm", [P, 640], dt.bfloat16))
        res = en(nc.sbuf_tensor("res", [1, 1], dt.float32))
        psum = [
            en(nc.psum_tensor(f"ps{i}", [P, CH], dt.float32)) for i in range(NCH)
        ]
        sum_ps = en(nc.psum_tensor("sum_ps", [1, 1], dt.float32))
        warm_ps = en(nc.psum_tensor("warm_ps", [P, CH], dt.float32))

        sem_ones = en(nc.semaphore("sem_ones"))
        sem_warm = en(nc.semaphore("sem_warm"))
        sem_w = en(nc.semaphore("sem_w"))      # PE warm-loop self ordering
        sem_aug = en(nc.semaphore("sem_aug"))
        sem_pen = en(nc.semaphore("sem_pen"))
        sem_pen2 = en(nc.semaphore("sem_pen2"))   # SWDGE sems must be exclusive
        sem_pe = en(nc.semaphore("sem_pe"))
        sem_v = en(nc.semaphore("sem_v"))      # DVE same-engine RAW ordering
        sem_a = en(nc.semaphore("sem_a"))      # ACT same-engine RAW ordering
        sem_dve = en(nc.semaphore("sem_dve"))
        sem_act = en(nc.semaphore("sem_act"))
        sem_sum = en(nc.semaphore("sem_sum"))
        sem_res = en(nc.semaphore("sem_res"))
        sem_out = en(nc.semaphore("sem_out"))
        block = en(nc.Block())

        @block.sync
        def _(s):
            s.dma_start(aug_s[0:3, :], aug[0:3, :]).then_inc(sem_aug, 16)
            s.dma_start(pen_s[0 : ROW_SPLIT[0], :], pen[0 : ROW_SPLIT[0], :]).then_inc(
                sem_pen, 16
            )
            s.wait_ge(sem_res, 1)
            s.dma_start(out[:], res[:]).then_inc(sem_out, 16)
            s.wait_ge(sem_out, 16)

        @block.scalar
        def _(a):
            a.dma_start(aug_s[3:5, :], aug[3:5, :]).then_inc(sem_aug, 16)
            a.dma_start(
                pen_s[ROW_SPLIT[0] : ROW_SPLIT[1], :],
                pen[ROW_SPLIT[0] : ROW_SPLIT[1], :],
            ).then_inc(sem_pen, 16)
            # dummy sqrt: pulls the sqrt LUT load to t~0, hidden under the DMAs
            a.wait_ge(sem_ones, 1)
            a.sqrt(stat[0:1, 8:9], ones[0:1, :])
            a.wait_ge(sem_dve, 1)
            a.sqrt(stat[:, 5:6], stat[:, 4:5]).then_inc(sem_a, 1)
            a.wait_ge(sem_a, 1)
            a.activation(stat[:, 7:8], stat[:, 5:6], AF.Tanh).then_inc(sem_act, 1)

        @block.gpsimd
        def _(g):
            g.memset(warm[:], 0.0).then_inc(sem_warm, 1)
            g.dma_start(
                pen_s[ROW_SPLIT[1] : P, :], pen[ROW_SPLIT[1] : P, :]
            ).then_inc(sem_pen2, 16)

        @block.tensor
        def _(t):
            # warm-up: ~2.8us of dummy bf16 matmuls flips the HAM clock gate
            # (1.2 -> 2.4 GHz) before the fp32 passes; PE is otherwise idle
            # while the aug DMA lands
            t.wait_ge(sem_warm, 1)
            for w in range(4):
                t.matmul(
                    warm_ps[:], warm[:, 0:P], warm[:, P : P + CH],
                    start=True, stop=True,
                ).then_inc(sem_w, 1)
                t.wait_ge(sem_w, w + 1)
            t.wait_ge(sem_aug, 32)
            for ch in range(NCH):
                t.matmul(
                    psum[ch][:], aug_s[:, 0:P],
                    aug_s[:, P + ch * CH : P + (ch + 1) * CH],
                    start=True, stop=True,
                ).then_inc(sem_pe, 1)
            t.wait_ge(sem_ones, 1)
            t.wait_ge(sem_act, 1)
            t.matmul(
                sum_ps[:], stat[:, 7:8], ones[:], start=True, stop=True
            ).then_inc(sem_sum, 1)

        @block.vector
        def _(v):
            c = 0
            v.memset(ones[:], 1.0).then_inc(sem_ones, 1)
            v.wait_ge(sem_pen, 32)
            v.wait_ge(sem_pen2, 16)
            for ch in range(NCH):
                sl = bass.ts(ch, CH)
                v.wait_ge(sem_pe, ch + 1)
                v.tensor_tensor(
                    out=scr[:, sl], in0=psum[ch][:], in1=pen_s[:, sl], op=OP.add
                ).then_inc(sem_v, 1)
                c += 1
                v.wait_ge(sem_v, c)
                v.tensor_reduce(
                    stat[:, ch : ch + 1], scr[:, sl], axis=AX.X, op=OP.min
                ).then_inc(sem_v, 1)
                c += 1
            v.wait_ge(sem_v, c)
            v.tensor_reduce(
                stat[:, 2:3], stat[:, 0:NCH], axis=AX.X, op=OP.min
            ).then_inc(sem_v, 1)
            c += 1
            v.wait_ge(sem_v, c)
            v.tensor_scalar(
                out=stat[:, 3:4], in0=stat[:, 2:3], scalar1=TAU, scalar2=None,
                op0=OP.is_ge,
            ).then_inc(sem_v, 1)
            c += 1
            v.wait_ge(sem_v, c)
            v.tensor_tensor(
                out=stat[:, 4:5], in0=stat[:, 2:3], in1=stat[:, 3:4], op=OP.mult
            ).then_inc(sem_dve, 1)
            v.wait_ge(sem_sum, 1)
            v.tensor_copy(res[:], sum_ps[:]).then_inc(sem_res, 1)

    nc.compile()
    return nc


def prepare_in_maps(presented_contact, vertices, geomask):
    pc = np.asarray(presented_contact).astype(np.int64)
    verts = np.asarray(vertices, dtype=np.float32).reshape(N, 3)
    gm = np.asarray(geomask)

    vc = verts[pc]                                    # [C, 3]
    q2 = (vc * vc).sum(axis=1, dtype=np.float32)      # [C]
    ones = np.ones((1, C), np.float32)
    ak = np.concatenate([vc.T, ones, q2[None, :]], axis=0).astype(np.float32)
    mg = gm[pc][:, pc]                                # [C, C] bool
    pen = np.where(mg, 0, 255).astype(np.uint8)   # 255 > max contact dist^2

    in_maps = []
    for g in range(NCORES):
        sl = slice(g * P, (g + 1) * P)
        aq = np.concatenate(
            [(-2.0 * vc[sl].T), q2[None, sl], np.ones((1, P), np.float32)], axis=0
        ).astype(np.float32)
        aug = np.concatenate([aq, ak], axis=1).astype(np.float32)   # [5, P+C]
        in_maps.append({
            "aug": np.ascontiguousarray(aug),
            "pen": np.ascontiguousarray(pen[sl]),
        })
    return in_maps


def finish(results) -> np.ndarray:
    sums = np.array([results[g]["out"][0, 0] for g in range(NCORES)], np.float64)
    return np.asarray(sums.sum() / C, dtype=np.float32)


def kernel(presented_contact, vertices, geomask) -> np.ndarray:
    in_maps = prepare_in_maps(presented_contact, vertices, geomask)
    nc = build_nc()
    res = run_bass_kernel_spmd(nc, in_maps, list(range(NCORES)))
    return finish(res.results)
